# revision 14
# baseline (speedup 1.0000x reference)
"""DynamicCenterLoss on Trainium2 (Bass, raw engine scheduling), 8-core SPMD.

Strategy: `batch` is sorted, so core b owns batch b (~N/8 points).
The wire format is a per-point fp8-e4m3 encoding built on the host:

    per point: [ onehot16(tgt) | feat (64) | 1 | ||feat||^2 / 16 ]

82 bytes/point (5.4 MB/core vs 17 MB in f32) packed tile-by-tile into
one DRAM tensor so each SBUF tile loads with a single DMA.  Loss
tolerance is 2e-2; the fp8 quantization costs 9.5e-4 (measured on the
real inputs).

All reductions run on-device through one PE pass: fp8 DoubleRow
matmuls contract 256 points each (lhsT = onehot pairs [128,2,16],
rhs = ext pairs [128,2,66]), accumulating OUT[16, 66] in PSUM:
per-class feature sums (cols 0:64), counts (col 64) and per-class
sum ||f||^2/16 (col 65).  The 16-wide one-hot satisfies DoubleRow's
16B-aligned LDWEIGHTS Ko-step.  The tiny pairwise-center hinge and
final divisions run on the host from the 8x[16,66] stats.

This version schedules engines by hand (no TileContext): the Tile
framework's fixed preamble + 283-semaphore teardown costs ~9.2 us of
measured exec time (an empty tile kernel measures 11.5 us).  Here the
whole sync graph is 4 semaphores: two DMA rings (FIFO completion),
DMA->PE per-tile waits, PE->DVE for the PSUM copy, DVE->sync for the
result store, and gpsimd semaphore clears to leave state at zero.
"""

import numpy as np
import ml_dtypes

import concourse.bass as bass
import concourse.bacc as bacc
from concourse import mybir
from concourse.bass_utils import run_bass_kernel_spmd

P = 128
D = 64
DE = D + 2  # [feat | 1 | hsq]
C = 13
C16 = 16  # one-hot width on the wire
REC = C16 + DE  # 82 bytes per point
B = 8
N_CORES = 8
MARGIN = 0.5
INTRA_W = 1.0
INTER_W = 1.0
LOSS_W = 0.01
IGNORE = -1
SQ_SCALE = 16.0  # hsq = ||f||^2 / SQ_SCALE (fits e4m3 nicely)

f32 = mybir.dt.float32
f8 = mybir.dt.float8e4

NP_F8 = ml_dtypes.float8_e4m3


def _splits(T: int):
    """Tile sizes: small first (early PE start), large later (few DMAs)."""
    assert T % 2 == 0
    szs = [16, 32, 64, 96, 128]
    splits = []
    t0 = 0
    while t0 < T:
        sz = min(szs[min(len(splits), len(szs) - 1)], T - t0)
        splits.append((t0, sz))
        t0 += sz
    assert all(tt % 2 == 0 for _, tt in splits)
    return splits


def build_nc(T: int) -> bass.Bass:
    splits = _splits(T)

    nc = bacc.Bacc("TRN2", target_bir_lowering=False)
    pk_h = nc.dram_tensor("pk", [P, T * REC], f8, kind="ExternalInput")
    out_h = nc.dram_tensor("out", [C16, DE], f32, kind="ExternalOutput")

    s_a = nc.alloc_semaphore("s_ring_a")
    s_b = nc.alloc_semaphore("s_ring_b")
    s_pe = nc.alloc_semaphore("s_pe")
    s_cp = nc.alloc_semaphore("s_cp")

    tiles = [
        nc.alloc_sbuf_tensor(f"tile{i}", [P, tt * REC], f8)
        for i, (t0, tt) in enumerate(splits)
    ]
    out_sb = nc.alloc_sbuf_tensor("out_sb", [C16, DE], f32)
    acc = nc.alloc_psum_tensor("acc", [C16, DE], f32)

    # ---- DMA issue: tiles alternate between the two HWDGE rings ----
    ring_pos = {}
    pos_cnt = [0, 0]
    for i, (t0, tt) in enumerate(splits):
        r = i % 2
        eng = nc.sync if r == 0 else nc.scalar
        eng.dma_start(
            out=tiles[i][:, :],
            in_=pk_h[:, t0 * REC : (t0 + tt) * REC],
        ).then_inc([s_a, s_b][r], 16)
        pos_cnt[r] += 1
        ring_pos[i] = (r, pos_cnt[r])

    # ---- PE: per tile wait for its DMA (ring FIFO), then DoubleRow
    # matmuls accumulating into one PSUM tile ----
    nsteps = T // 2
    step = 0
    mm = None
    for i, (t0, tt) in enumerate(splits):
        r, pos = ring_pos[i]
        nc.tensor.wait_ge([s_a, s_b][r], 16 * pos)
        ohv = tiles[i][:, : tt * C16].rearrange("p (t c) -> p t c", c=C16)
        extv = tiles[i][:, tt * C16 :].rearrange("p (t d) -> p t d", d=DE)
        for t in range(0, tt, 2):
            mm = nc.tensor.matmul(
                acc[:, :],
                lhsT=ohv[:, t : t + 2, :],
                rhs=extv[:, t : t + 2, :],
                start=(step == 0),
                stop=(step == nsteps - 1),
                perf_mode=mybir.MatmulPerfMode.DoubleRow,
            )
            step += 1
    mm.then_inc(s_pe, 1)

    # ---- DVE: PSUM -> SBUF once PE is done ----
    nc.vector.wait_ge(s_pe, 1)
    nc.vector.tensor_copy(out_sb[:, :], acc[:, :]).then_inc(s_cp, 1)

    # ---- result store + semaphore reset (repeatability) ----
    nc.sync.wait_ge(s_cp, 1)
    nc.sync.dma_start(out=out_h[:, :], in_=out_sb[:, :]).then_inc(s_a, 16)
    nc.gpsimd.wait_ge(s_a, 16 * (pos_cnt[0] + 1))
    for s in (s_a, s_b, s_pe, s_cp):
        nc.gpsimd.sem_clear(s)

    nc.finalize()
    return nc


# set by test.py to capture profile info
TRACE = False
LAST = {}


def _ensure_ntff_hook():
    """The agent image's antenv lacks axon_hooks; synthesize it so
    run_bass_kernel_spmd(trace=True) can profile. Best-effort."""
    import sys
    import types

    try:
        from antenv.axon_hooks import get_axon_ntff_profile_hook  # noqa: F401
        return
    except ImportError:
        pass
    try:
        from trn_agent_boot.trn_boot import _ntff_profile_via_ctypes

        hook = _ntff_profile_via_ctypes("/opt/axon/libaxon_pjrt.so")
        mod = types.ModuleType("antenv.axon_hooks")
        mod._hook = hook
        mod.get_axon_ntff_profile_hook = lambda: mod._hook
        mod.set_axon_ntff_profile_hook = lambda h: setattr(mod, "_hook", h)
        sys.modules["antenv.axon_hooks"] = mod
        import antenv

        antenv.axon_hooks = mod
    except Exception as e:  # degrade: no profile, run still works
        print(f"ntff hook injection failed: {e}")


def kernel(pred=None, target=None, feat=None, batch=None, centers=None):
    target = np.asarray(target)
    feat = np.asarray(feat, dtype=np.float32)
    batch = np.asarray(batch)
    centers = np.asarray(centers, dtype=np.float64)

    # shard at batch boundaries: core b <- batch b (batch is sorted)
    bounds = np.searchsorted(batch, np.arange(B + 1))
    sizes = np.diff(bounds)
    T = int(max((int(sizes.max()) + P - 1) // P, 16))
    T += T % 2  # DoubleRow matmuls consume point-pairs
    Npad = P * T
    splits = _splits(T)

    feat8 = feat.astype(NP_F8)
    hsq8 = ((feat8.astype(np.float32) ** 2).sum(1) / SQ_SCALE).astype(NP_F8)
    in_maps = []
    for b in range(B):
        lo, hi = int(bounds[b]), int(bounds[b + 1])
        n = hi - lo
        ext = np.zeros((Npad, DE), dtype=NP_F8)
        ext[:n, :D] = feat8[lo:hi]
        ext[:n, D] = np.asarray(1.0, dtype=NP_F8)
        ext[:n, D + 1] = hsq8[lo:hi]
        oh = np.zeros((Npad, C16), dtype=NP_F8)
        tb = target[lo:hi]
        valid = tb != IGNORE
        oh[:n] = (tb[:, None] == np.arange(C16, dtype=tb.dtype)).astype(NP_F8)
        if not valid.all():
            oh[:n][~valid] = np.asarray(0.0, dtype=NP_F8)
            ext[:n][~valid] = np.asarray(0.0, dtype=NP_F8)
        # pack per tile: [P, tt*16] one-hot block then [P, tt*66] ext block
        ohr = oh.reshape(P, T, C16)
        extr = ext.reshape(P, T, DE)
        blocks = []
        for t0, tt in splits:
            blocks.append(ohr[:, t0 : t0 + tt].reshape(P, tt * C16))
            blocks.append(extr[:, t0 : t0 + tt].reshape(P, tt * DE))
        pk = np.ascontiguousarray(np.concatenate(blocks, axis=1))
        in_maps.append({"pk": pk})

    nc = build_nc(T)
    if TRACE:
        _ensure_ntff_hook()
    res = run_bass_kernel_spmd(nc, in_maps, list(range(N_CORES)), trace=TRACE)
    LAST["results"] = res

    # ---- host finale (tiny: 8 cores x [16, 66] stats) ----
    intra_sum = 0.0
    inter_sum = 0.0
    present_cnt = 0
    cn2 = (centers ** 2).sum(1)  # (13,)
    for b in range(B):
        o = np.asarray(res.results[b]["out"]).astype(np.float64)  # [16, 66]
        fsum = o[:C, :D]  # (13, 64)
        ccnt = o[:C, D]  # (13,)
        S = SQ_SCALE * o[:C, D + 1].sum()
        cnt_b = ccnt.sum()
        if cnt_b <= 0:
            continue
        present_cnt += 1
        # intra: S - 2 sum_c c.fsum + sum_c ccnt*||c||^2, / cnt
        tdot = float((centers * fsum).sum())
        utot = float((ccnt * cn2).sum())
        intra_sum += (S - 2.0 * tdot + utot) / cnt_b
        # inter: pairwise hinge on class means
        pres = ccnt > 0
        cm = fsum / np.maximum(ccnt, 1.0)[:, None]
        diff = cm[:, None, :] - cm[None, :, :]
        dd2 = (diff ** 2).sum(-1)
        eye = np.eye(C, dtype=bool)
        pm = pres[:, None] & pres[None, :] & ~eye
        dist = np.sqrt(np.where(pm, dd2, 1.0))
        terms = np.where(pm, np.maximum(MARGIN - dist, 0.0), 0.0)
        npairs = pm.sum()
        inter_sum += terms.sum() / max(npairs, 1)

    den = max(present_cnt, 1)
    loss = LOSS_W * (INTRA_W * intra_sum / den + INTER_W * inter_sum / den)
    return np.float32(loss)
